# revision 1
# baseline (speedup 1.0000x reference)
"""RWKV-4 WKV attention layer on 8 TRN2 NeuronCores.

Reference computation (T=4096, NE=DA=2048, fp32):
    xx  = shift(x)  (zero-pad first row)
    xk/xv/xr = lerp(xx, x, time_mix_*)
    k, v, r = xk @ Wk, xv @ Wv, xr @ Wr
    wkv = serial scan over T with per-channel decay w = -exp(time_decay),
          bonus u = time_first
    out = (sigmoid(r) * wkv) @ Wo

Distribution strategy:
  - T-shard the projections: core i owns tokens [512i, 512(i+1)); it
    DMA-transposes its x slice to [NE, T] layout (bf16 xbar transpose),
    does the time-mix on DVE, and computes k/v/r for ALL channels with
    activations as the matmul stationary operand.
  - AllToAll #1 re-shards k/v/r by channel: core i ends with
    [256 channels, all 4096 tokens].
  - The WKV scan uses the unstabilized linear recurrence
        S_t = lam * S_{t-1} + e^{k_t} (*v_t),   lam = exp(-exp(time_decay))
    which is numerically safe in fp32 for this input distribution
    (k ~ N(0,1)), computed with the HW tensor_tensor_scan instruction
    along the free (time) axis. y_t = (S_{t-1}^{num} + e^u e^{k_t} v_t) /
    (S_{t-1}^{den} + e^u e^{k_t}).
  - AllToAll #2 re-shards A = sigmoid(r)*wkv back to T-sharding, giving
    each core A^T [2048, 512] which feeds the output matmul as the
    stationary operand; out slice [512, 2048] fp32 is DMA'd out.
  - Host concatenates the 8 output slices.
"""

import math
import os
import sys
from contextlib import ExitStack

for _p in ("/opt/trn_rl_repo", "/root/.axon_site/_ro/trn_rl_repo"):
    if os.path.isdir(_p) and _p not in sys.path:
        sys.path.insert(0, _p)

import numpy as np
import ml_dtypes

import concourse.bass as bass
import concourse.tile as tile
from concourse import bacc, mybir
from concourse.bass_utils import run_bass_kernel_spmd

F32 = mybir.dt.float32
BF16 = mybir.dt.bfloat16
AL = mybir.AluOpType
ACTF = mybir.ActivationFunctionType
P = 128


class Cfg:
    def __init__(self, T=4096, NE=2048, DA=2048, NC=8, TH=2048):
        self.T, self.NE, self.DA, self.NC = T, NE, DA, NC
        self.TSL = T // NC          # tokens per core
        self.CSL = DA // NC         # channels per core
        self.NKT = NE // P          # contraction tiles (projections)
        self.NMT = self.TSL // P    # T tiles per slice
        self.NNT = DA // 512        # N tiles (projections)
        self.NCT = self.CSL // P    # channel ptiles per core
        self.NKT2 = DA // P         # contraction tiles (output matmul)
        self.NOT = NE // 512        # N tiles (output matmul)
        self.TH = min(TH, T)        # scan T-half size
        self.NH = T // self.TH      # number of scan chunks
        assert self.TSL % P == 0 and self.CSL % P == 0
        assert DA % 512 == 0 and NE % 512 == 0 and T % self.TH == 0


def _bcast(ap, n):
    """[P,1] AP -> [P,n] stride-0 broadcast along free."""
    return bass.AP(ap.tensor, ap.offset, [ap.ap[0], [0, n]])


def build_kernel(cfg: Cfg, no_cc: bool = False, reps: int = 1, ablate: str | None = None):
    nc = bacc.Bacc("TRN2", target_bir_lowering=False, debug=False,
                   num_devices=1 if no_cc else cfg.NC)

    def _collective(kind, op, replica_groups, ins, outs):
        if no_cc:
            nc.gpsimd.dma_start(out=outs[0], in_=ins[0])
        else:
            nc.gpsimd.collective_compute(kind, op, replica_groups=replica_groups,
                                         ins=ins, outs=outs)
    T, NE, DA, NC = cfg.T, cfg.NE, cfg.DA, cfg.NC
    TSL, CSL = cfg.TSL, cfg.CSL
    TH, NH = cfg.TH, cfg.NH
    RG = [list(range(NC))]
    XW = TSL + P                      # x^T chunk width (with halo)

    xs = nc.declare_dram_parameter("xs", [TSL + P, NE], BF16, isOutput=False)
    NG = DA // 512
    wk = nc.declare_dram_parameter("wk", [NG * P, cfg.NKT * 512], BF16, isOutput=False)
    wv = nc.declare_dram_parameter("wv", [NG * P, cfg.NKT * 512], BF16, isOutput=False)
    wr = nc.declare_dram_parameter("wr", [NG * P, cfg.NKT * 512], BF16, isOutput=False)
    wo = nc.declare_dram_parameter("wo", [cfg.NOT * P, cfg.NKT2 * 512], BF16, isOutput=False)
    tmk = nc.declare_dram_parameter("tmk", [P, cfg.NKT], F32, isOutput=False)
    tmv = nc.declare_dram_parameter("tmv", [P, cfg.NKT], F32, isOutput=False)
    tmr = nc.declare_dram_parameter("tmr", [P, cfg.NKT], F32, isOutput=False)
    lam = nc.declare_dram_parameter("lam", [P, cfg.NCT], F32, isOutput=False)
    eu = nc.declare_dram_parameter("eu", [P, cfg.NCT], F32, isOutput=False)
    out = nc.declare_dram_parameter("out", [TSL, NE], F32, isOutput=True)

    projs = [("k", wk, tmk), ("v", wv, tmv), ("r", wr, tmr)]

    with tile.TileContext(nc) as tc, ExitStack() as octx:
        dram = octx.enter_context(tc.tile_pool(name="dram", bufs=1, space="DRAM"))
        psum = octx.enter_context(tc.tile_pool(name="psum", bufs=8, space="PSUM"))
        const_pool = octx.enter_context(tc.tile_pool(name="const", bufs=1))
        persist = octx.enter_context(tc.tile_pool(name="persist", bufs=1))
        tokp = octx.enter_context(tc.tile_pool(name="tokp", bufs=2))

        # small constants
        tm_sb = {}
        for name, src in (("k", tmk), ("v", tmv), ("r", tmr)):
            t = const_pool.tile([P, cfg.NKT], F32, tag=f"tm{name}", name=f"tm{name}_sb")
            nc.sync.dma_start(t[:], src[:])
            tm_sb[name] = t
        lam_sb = const_pool.tile([P, cfg.NCT], F32, tag="lam")
        nc.sync.dma_start(lam_sb[:], lam[:])
        eu_sb = const_pool.tile([P, cfg.NCT], F32, tag="eu")
        nc.sync.dma_start(eu_sb[:], eu[:])

        # DRAM bounce buffers for the collectives (shared across reps).
        # Each exchange is split into NCT per-ptile collectives: half h of
        # tensor X carries every rank's ptile-h channel block, so consumers
        # of ptile h unblock after only that half's exchange.
        HDA = NC * P                       # rows per half buffer
        a2a_in = {}
        a2a_out = {}
        for name, _, _ in projs:
            a2a_in[name] = [dram.tile([HDA, TSL], BF16, tag=f"ai_{name}{h}",
                                      name=f"ai_{name}{h}") for h in range(cfg.NCT)]
            a2a_out[name] = [dram.tile([HDA, TSL], BF16, tag=f"ao_{name}{h}",
                                       name=f"ao_{name}{h}") for h in range(cfg.NCT)]
        a2a_in_a = [dram.tile([HDA, TSL], BF16, tag=f"ai_a{h}", name=f"ai_a{h}")
                    for h in range(cfg.NCT)]
        a2a_out_a = [dram.tile([HDA, TSL], BF16, tag=f"ao_a{h}", name=f"ao_a{h}")
                     for h in range(cfg.NCT)]

        # post-A2A channel-sharded tensors [P, T] (bf16), per channel-ptile
        kvrT = {name: [persist.tile([P, T], BF16, tag=f"{name}T{pt}", name=f"{name}T{pt}")
                       for pt in range(cfg.NCT)] for name, _, _ in projs}
        a16 = [persist.tile([P, T], BF16, tag=f"a16_{pt}", name=f"a16_{pt}")
               for pt in range(cfg.NCT)]

        prev_osts = None
        for rep in range(reps):
            prev_osts = _emit_body(
                nc, tc, cfg, rep, projs, tm_sb, lam_sb, eu_sb,
                a2a_in, a2a_out, a2a_in_a, a2a_out_a, kvrT, a16,
                xs, wo, out, psum, _collective, RG, ablate, tokp, prev_osts)

    nc.finalize()
    return nc


def _make_token(nc, tokp, osts, R):
    """Tiny persistent tile whose value depends on all final staging tiles —
    the next rep's gate reads it to serialize bodies for timing."""
    tok = tokp.tile([1, 8], bass.mybir.dt.float32, tag="tok", name=R + "tok")
    for i, o in enumerate(osts):
        nc.vector.tensor_copy(tok[0:1, 2 * (i % 4):2 * (i % 4) + 2],
                              o[0:1, 0:2])
    return tok


def _emit_body(nc, tc, cfg, rep, projs, tm_sb, lam_sb, eu_sb,
               a2a_in, a2a_out, a2a_in_a, a2a_out_a, kvrT, a16,
               xs, wo, out, psum, _collective, RG, ablate=None,
               tokp=None, prev_osts=None):
    T, NE, DA, NC = cfg.T, cfg.NE, cfg.DA, cfg.NC
    TSL, CSL, TH, NH = cfg.TSL, cfg.CSL, cfg.TH, cfg.NH
    XW = TSL + P
    R = f"r{rep}_"

    # ---------------- phase A: transpose x, time-mix, projections ----------
    with tc.tile_pool(name=R + "phA", bufs=1) as phA, \
         tc.tile_pool(name=R + "gate", bufs=1) as gatep, \
         tc.tile_pool(name=R + "mixp", bufs=2) as mixp, \
         tc.tile_pool(name=R + "wstp", bufs=2) as wstp, \
         tc.tile_pool(name=R + "slabp", bufs=2) as slabp:

        # one transposed load of the whole x slice: chunk kt holds
        # x^T rows [128kt, 128(kt+1)) as columns [XW*kt, XW*(kt+1))
        xtrb = phA.tile([P, cfg.NKT * XW], BF16, tag="xtrb", name=R + "xtrb")
        if rep > 0:
            # serialize reps: scribble a token read from the previous rep's
            # final SBUF staging tiles (tracked dependency) into the tile
            # the transpose then overwrites
            nc.vector.tensor_copy(xtrb[0:1, 0:8], prev_osts[0:1, 0:8])
        b = xtrb[:, :]
        out3 = bass.AP(b.tensor, b.offset, [b.ap[0], [XW, cfg.NKT], [1, XW]])
        nc.sync.dma_start(out3, xs[:, :], transpose=True)
        xtr = [xtrb[:, XW * kt: XW * (kt + 1)] for kt in range(cfg.NKT)]

        # time-mix: xk = xx + tm*(x - xx) per NE-ptile
        mixes = {"k": [], "v": [], "r": []}
        for kt in range(cfg.NKT):
            xm = xtr[kt][:, P:XW]
            xx = xtr[kt][:, P - 1:XW - 1]
            d = mixp.tile([P, TSL], BF16, tag="d")
            nc.vector.tensor_sub(d[:], xm, xx)
            for name in ("k", "v", "r"):
                mt_ = phA.tile([P, TSL], BF16, tag=f"mx{name}{kt}",
                               name=R + f"mx{name}{kt}")
                nc.vector.scalar_tensor_tensor(
                    mt_[:], d[:], tm_sb[name][:, kt:kt + 1], xx,
                    op0=AL.mult, op1=AL.add)
                mixes[name].append(mt_)

        # projections: psum [128(C), TSL] per channel tile, groups of 4
        NG = DA // 512
        for name, wdram, _ in projs:
            for g in range(NG):
                wt = wstp.tile([P, cfg.NKT * 512], BF16, tag="wst",
                               name=R + f"w_{name}_{g}")
                nc.scalar.dma_start(wt[:], wdram[P * g: P * (g + 1), :])
                pts = [psum.tile([P, TSL], F32, tag="pp",
                                 name=R + f"ps_{name}_{g}_{c4}")
                       for c4 in range(4)]
                for kt in range(cfg.NKT):
                    for c4 in range(4):
                        nc.tensor.matmul(
                            pts[c4][:],
                            wt[:, kt * 512 + 128 * c4: kt * 512 + 128 * (c4 + 1)],
                            mixes[name][kt][:, :],
                            start=(kt == 0), stop=(kt == cfg.NKT - 1))
                slab = slabp.tile([P, 4 * TSL], BF16, tag="slab",
                                  name=R + f"sl_{name}_{g}")
                for c4 in range(4):
                    if name == "r":
                        # fold sigmoid into the PSUM drain: the exchange
                        # carries sigmoid(r) directly
                        nc.scalar.activation(slab[:, TSL * c4: TSL * (c4 + 1)],
                                             pts[c4][:], ACTF.Sigmoid)
                    else:
                        nc.scalar.copy(slab[:, TSL * c4: TSL * (c4 + 1)],
                                       pts[c4][:])
                # c4 block holds channels [512g+128c4, ...): destination
                # rank j = 2g + c4//2, ptile-half h = c4 % NCT
                NJ = (4 * P) // CSL       # ranks covered per slab group
                for h in range(cfg.NCT):
                    dsth = a2a_in[name][h][:]
                    dst3 = bass.AP(dsth.tensor,
                                   dsth.offset + (NJ * g) * P * TSL,
                                   [[TSL, P], [P * TSL, NJ], [1, TSL]])
                    sb = slab[:, :]
                    src3 = bass.AP(sb.tensor, sb.offset + h * TSL,
                                   [sb.ap[0], [cfg.NCT * TSL, NJ], [1, TSL]])
                    nc.sync.dma_start(dst3, src3)

            for h in range(cfg.NCT):
                _collective(
                    "AllToAll", AL.bypass, replica_groups=RG,
                    ins=[a2a_in[name][h][:].opt()],
                    outs=[a2a_out[name][h][:].opt()])

            for pt in range(cfg.NCT):
                s = a2a_out[name][pt][:]
                src3 = bass.AP(s.tensor, s.offset,
                               [[TSL, P], [P * TSL, NC], [1, TSL]])
                nc.scalar.dma_start(kvrT[name][pt][:], src3)

    if ablate == "A":
        with tc.tile_pool(name=R + "abl", bufs=1) as ablp:
            osts = []
            srcs = [kvrT[n][pt_][:, 0:NE] for n in ("k", "v", "r")
                    for pt_ in range(cfg.NCT)]
            for mt in range(cfg.NMT):
                o = ablp.tile([P, NE], F32, tag=f"ao{mt}", name=R + f"ablo{mt}")
                nc.scalar.copy(o[:], srcs[mt % len(srcs)])
                nc.sync.dma_start(out[P * mt: P * (mt + 1), :], o[:])
                osts.append(o)
            tok = _make_token(nc, tokp, osts, R)
        return tok

    # ---------------- phase B: WKV scan (channel-sharded) ------------------
    with tc.tile_pool(name=R + "phB", bufs=2) as phB:
        for pt in range(cfg.NCT):
            lam_b = _bcast(lam_sb[:, pt:pt + 1], TH)
            eu_ap = eu_sb[:, pt:pt + 1]
            prevP = prevQ = None
            for h in range(NH):
                ts_, te = h * TH, (h + 1) * TH
                ek = phB.tile([P, TH], BF16, tag="ek")
                nc.scalar.activation(ek[:], kvrT["k"][pt][:, ts_:te], ACTF.Exp)
                ekv = phB.tile([P, TH], BF16, tag="ekv")
                nc.gpsimd.tensor_mul(ekv[:], ek[:], kvrT["v"][pt][:, ts_:te])

                Pst = phB.tile([P, TH + 1], BF16, tag="Pst")
                Qst = phB.tile([P, TH + 1], BF16, tag="Qst")
                if h == 0:
                    nc.gpsimd.memset(Pst[:, 0:1], 0.0)
                    nc.gpsimd.memset(Qst[:, 0:1], 0.0)
                else:
                    nc.gpsimd.tensor_copy(Pst[:, 0:1], prevP[:, TH:TH + 1])
                    nc.gpsimd.tensor_copy(Qst[:, 0:1], prevQ[:, TH:TH + 1])
                nc.vector.tensor_tensor_scan(
                    Pst[:, 1:TH + 1], lam_b, ekv[:], Pst[:, 0:1],
                    op0=AL.mult, op1=AL.add)
                nc.vector.tensor_tensor_scan(
                    Qst[:, 1:TH + 1], lam_b, ek[:], Qst[:, 0:1],
                    op0=AL.mult, op1=AL.add)

                # num (bf16, 2x) -> ekv; den must be fp32 for the reciprocal
                den = phB.tile([P, TH], F32, tag="den")
                nc.vector.scalar_tensor_tensor(
                    ekv[:], ekv[:], eu_ap, Pst[:, 0:TH],
                    op0=AL.mult, op1=AL.add)
                nc.vector.scalar_tensor_tensor(
                    den[:], ek[:], eu_ap, Qst[:, 0:TH],
                    op0=AL.mult, op1=AL.add)
                nc.vector.reciprocal_approx_fast(den[:], den[:])
                nc.vector.tensor_mul(Pst[:, 0:TH], ekv[:], den[:])
                nc.vector.tensor_mul(a16[pt][:, ts_:te], Pst[:, 0:TH],
                                     kvrT["r"][pt][:, ts_:te])
                prevP, prevQ = Pst, Qst

        for pt in range(cfg.NCT):
            dst = a2a_in_a[pt][:]
            dst3 = bass.AP(dst.tensor, dst.offset,
                           [[TSL, P], [P * TSL, NC], [1, TSL]])
            nc.scalar.dma_start(dst3, a16[pt][:, :])
            _collective(
                "AllToAll", AL.bypass, replica_groups=RG,
                ins=[a2a_in_a[pt][:].opt()], outs=[a2a_out_a[pt][:].opt()])

    if ablate == "AB":
        with tc.tile_pool(name=R + "abl", bufs=1) as ablp:
            osts = []
            for mt in range(cfg.NMT):
                o = ablp.tile([P, NE], F32, tag=f"ao{mt}", name=R + f"ablo{mt}")
                nc.scalar.copy(o[:], a16[mt % cfg.NCT][:, 0:NE])
                nc.sync.dma_start(out[P * mt: P * (mt + 1), :], o[:])
                osts.append(o)
            tok = _make_token(nc, tokp, osts, R)
        return tok

    # ---------------- phase C: output matmul -------------------------------
    with tc.tile_pool(name=R + "phC", bufs=1) as phC, \
         tc.tile_pool(name=R + "wop", bufs=2) as wop, \
         tc.tile_pool(name=R + "ostl", bufs=1) as ostl:
        # atb channel block kt2 = NCT*j + h comes from half h, rank j
        atb = phC.tile([P, cfg.NKT2 * TSL], BF16, tag="atb", name=R + "atb")
        for h in range(cfg.NCT):
            ab = atb[:, :]
            dst3 = bass.AP(ab.tensor, ab.offset + h * TSL,
                           [ab.ap[0], [cfg.NCT * TSL, NC], [1, TSL]])
            s = a2a_out_a[h][:]
            src3 = bass.AP(s.tensor, s.offset,
                           [[TSL, P], [P * TSL, NC], [1, TSL]])
            nc.scalar.dma_start(dst3, src3)
        osts = [ostl.tile([P, NE], F32, tag=f"ost{mt}", name=R + f"ost{mt}")
                for mt in range(cfg.NMT)]
        # nt pairs share each stationary load (halves LDWEIGHTS)
        for ntp in range(cfg.NOT // 2):
            nts = (2 * ntp, 2 * ntp + 1)
            wots = []
            for nt in nts:
                wot = wop.tile([P, cfg.NKT2 * 512], BF16, tag="wo",
                               name=R + f"wo_{nt}")
                nc.scalar.dma_start(wot[:], wo[P * nt: P * (nt + 1), :])
                wots.append(wot)
            pts = {(mt_, i_): psum.tile([P, 512], F32, tag="pp",
                                        name=R + f"po_{ntp}_{mt_}_{i_}")
                   for mt_ in range(cfg.NMT) for i_ in range(2)}
            kt_order = [cfg.NCT * j + h for h in range(cfg.NCT) for j in range(NC)]
            for ki, kt in enumerate(kt_order):
                for mt in range(cfg.NMT):
                    lhsT = atb[:, kt * TSL + P * mt: kt * TSL + P * (mt + 1)]
                    for i_ in range(2):
                        nc.tensor.matmul(
                            pts[(mt, i_)][:], lhsT,
                            wots[i_][:, 512 * kt: 512 * (kt + 1)],
                            start=(ki == 0), stop=(ki == cfg.NKT2 - 1))
            for mt in range(cfg.NMT):
                for i_ in range(2):
                    nt = nts[i_]
                    nc.scalar.copy(osts[mt][:, 512 * nt: 512 * (nt + 1)],
                                   pts[(mt, i_)][:])
        for mt in range(cfg.NMT):
            nc.sync.dma_start(out[P * mt: P * (mt + 1), :], osts[mt][:])
        tok = _make_token(nc, tokp, osts, R)
    return tok


# ------------------------------------------------------------------------
# host side
# ------------------------------------------------------------------------

_CACHE = {}


def _get_nc(cfg: Cfg):
    key = (cfg.T, cfg.NE, cfg.DA, cfg.NC, cfg.TH)
    if key not in _CACHE:
        _CACHE[key] = build_kernel(cfg)
    return _CACHE[key]


def make_in_maps(cfg: Cfg, x, time_first, time_decay, time_mix_k, time_mix_v,
                 time_mix_r, W_key, W_value, W_receptance, W_output):
    T, NE, DA, NC = cfg.T, cfg.NE, cfg.DA, cfg.NC
    TSL, CSL = cfg.TSL, cfg.CSL
    bf = ml_dtypes.bfloat16

    x = np.asarray(x, np.float32)
    xpad = np.zeros((P + T, NE), bf)
    xpad[P:] = x.astype(bf)

    def tile_w(w, nkt, ng):
        # [NE, DA] -> [NG*P, NKT*512]: strip g rows hold W[128kt+p, 512g+c]
        w = np.asarray(w, np.float32).astype(bf)
        return np.ascontiguousarray(
            w.reshape(nkt, P, ng, 512).transpose(2, 1, 0, 3)
            .reshape(ng * P, nkt * 512))

    wk16 = tile_w(W_key, cfg.NKT, DA // 512)
    wv16 = tile_w(W_value, cfg.NKT, DA // 512)
    wr16 = tile_w(W_receptance, cfg.NKT, DA // 512)
    wo16 = tile_w(W_output, cfg.NKT2, cfg.NOT)

    def col_fold(v, n_t):  # [n_t*P] -> [P, n_t]
        return np.ascontiguousarray(
            np.asarray(v, np.float64).reshape(-1)[: n_t * P]
            .reshape(n_t, P).T.astype(np.float32))

    tmk_a = col_fold(time_mix_k, cfg.NKT)
    tmv_a = col_fold(time_mix_v, cfg.NKT)
    tmr_a = col_fold(time_mix_r, cfg.NKT)

    td = np.asarray(time_decay, np.float64).reshape(-1)
    lam_full = np.exp(-np.exp(td))
    eu_full = np.exp(np.asarray(time_first, np.float64).reshape(-1))

    in_maps = []
    for i in range(NC):
        xsl = np.ascontiguousarray(xpad[TSL * i: TSL * i + TSL + P, :])
        lam_i = np.ascontiguousarray(
            lam_full[CSL * i: CSL * (i + 1)].reshape(cfg.NCT, P).T
            .astype(np.float32))
        eu_i = np.ascontiguousarray(
            eu_full[CSL * i: CSL * (i + 1)].reshape(cfg.NCT, P).T
            .astype(np.float32))
        in_maps.append({
            "xs": xsl, "wk": wk16, "wv": wv16, "wr": wr16, "wo": wo16,
            "tmk": tmk_a, "tmv": tmv_a, "tmr": tmr_a,
            "lam": lam_i, "eu": eu_i,
        })
    return in_maps


def kernel(x, time_first, time_decay, time_mix_k, time_mix_v, time_mix_r,
           W_key, W_value, W_receptance, W_output, _trace=False):
    cfg = Cfg(T=int(np.asarray(x).shape[0]), NE=int(np.asarray(x).shape[1]),
              DA=int(np.asarray(time_decay).reshape(-1).shape[0]), NC=8)
    nc = _get_nc(cfg)
    in_maps = make_in_maps(cfg, x, time_first, time_decay, time_mix_k,
                           time_mix_v, time_mix_r, W_key, W_value,
                           W_receptance, W_output)
    res = run_bass_kernel_spmd(nc, in_maps, core_ids=list(range(cfg.NC)),
                               trace=_trace)
    outp = np.concatenate([res.results[i]["out"] for i in range(cfg.NC)], axis=0)
    out_final = outp.astype(np.float32)
    if _trace:
        return out_final, res
    return out_final

